# revision 24
# baseline (speedup 1.0000x reference)
"""Trainium2 Bass kernel for nn_Aggregator (GNN message passing).

Computation per (b, e):
  scores[k] = <side[b], rel[b,e,k,:]>          (contract over D=64)
  attn      = softmax_k(scores)
  agg[d]    = sum_k attn[k] * nbr[b,e,k,d]     (contract over K=32)
  out       = relu(cat(self[b,e], agg) @ W + bias)

Sharding: data-parallel over the leading batch dim B=1024 across 8 cores
(128 batches/core); weights replicated.

Per-core mapping, fully unrolled over 32 "bgroups" of 4 batches with all
big tiles on (4b x 32e) partitions:
  - rel and nbr^T arrive host-packed in one tensor (one sequential DMA/j)
  - scores/agg are DVE broadcast-multiply + innermost-axis reduce pairs
  - softmax: ACT exp, DVE row-sum + reciprocal; the 1/sum is folded into
    a single post-scale of the unnormalized aggregate
  - linear: self^T arrives host-transposed; agg^T via one PE transpose;
    out = self@W1 + agg@W2 + 1x128 rank-1 bias, PSUM-accumulated on PE;
    relu on ACT
  - the loop body is software-pipelined: scores(j) is emitted before the
    softmax/agg/linear of j-1, so ACT/PE latency hides under DVE work
"""

import numpy as np

B, E, K, D = 1024, 32, 32, 64
NCORES = 8
BC = B // NCORES  # 128 batches per core
NJ = BC // 4      # 32 bgroups of 4 batches

_CACHE = {}


def _build_nc():
    from contextlib import ExitStack

    import concourse.bass as bass
    import concourse.bacc as bacc
    import concourse.tile as tile
    from concourse import mybir

    f32 = mybir.dt.float32
    Alu = mybir.AluOpType
    Act = mybir.ActivationFunctionType

    # Bacc (not raw Bass): its finalize() legalizes sync waits -- TRN2 allows
    # at most 1 wait per instruction; excess waits split into EventSemaphores.
    nc = bacc.Bacc()

    # rel and nbr^T packed along the last axis: rn[b,e,0:2048] = rel[b,e]
    # (k-major), rn[b,e,2048:4096] = nbr[b,e]^T (d-major) -- one DMA per j.
    rn_h = nc.declare_dram_parameter("rn", [BC, E, 2 * K * D], f32, isOutput=False)
    selft_h = nc.declare_dram_parameter("selft", [D, BC, E], f32, isOutput=False)
    side_h = nc.declare_dram_parameter("side", [BC, D], f32, isOutput=False)
    w_h = nc.declare_dram_parameter("wmat", [2 * D, D], f32, isOutput=False)
    b_h = nc.declare_dram_parameter("bvec", [1, D], f32, isOutput=False)
    ones_h = nc.declare_dram_parameter("ones", [1, 128], f32, isOutput=False)
    iden_h = nc.declare_dram_parameter("iden", [128, 128], f32, isOutput=False)
    out_h = nc.declare_dram_parameter("out", [BC, E, D], f32, isOutput=True)

    rn_ap = rn_h[:]
    selft_ap = selft_h[:]
    out_ap = out_h[:]

    with tile.TileContext(nc) as tc, ExitStack() as ctx:
        consts = ctx.enter_context(tc.tile_pool(name="consts", bufs=1))
        bigrn = ctx.enter_context(tc.tile_pool(name="bigrn", bufs=5))
        prods = ctx.enter_context(tc.tile_pool(name="prods", bufs=2))
        work = ctx.enter_context(tc.tile_pool(name="work", bufs=4))
        # scores intermediate in PSUM: DVE's PSUM ports are separate from its
        # SBUF ports
        ps_prod = ctx.enter_context(tc.tile_pool(name="ps_prod", bufs=1, space="PSUM"))
        ps_at = ctx.enter_context(tc.tile_pool(name="ps_at", bufs=2, space="PSUM"))
        ps_lin = ctx.enter_context(tc.tile_pool(name="ps_lin", bufs=2, space="PSUM"))

        w1_sb = consts.tile([D, D], f32)
        nc.sync.dma_start(out=w1_sb, in_=w_h[:][0:D])
        w2_sb = consts.tile([D, D], f32)
        nc.sync.dma_start(out=w2_sb, in_=w_h[:][D : 2 * D])
        bvec_sb = consts.tile([1, D], f32)
        nc.sync.dma_start(out=bvec_sb, in_=b_h[:])
        ones_sb = consts.tile([1, 128], f32)
        nc.sync.dma_start(out=ones_sb, in_=ones_h[:])
        iden_sb = consts.tile([128, 128], f32)
        nc.sync.dma_start(out=iden_sb, in_=iden_h[:])
        # side_all[p=(bg,e), j, d] = side[4j + bg, d]; loaded once, one DMA
        # per bg-row block (3-dim AP limit).
        side_all = consts.tile([128, NJ, D], f32)
        for bg in range(4):
            nc.sync.dma_start(
                out=side_all[32 * bg : 32 * bg + 32],
                in_=bass.AP(
                    tensor=side_h[:].tensor,
                    offset=bg * D,
                    ap=[[0, 32], [4 * D, NJ], [1, D]],
                ),
            )

        # pipeline state carried from stage A (scores) to stage B (rest)
        st = {}

        def stage_a(j):
            rn_sb = bigrn.tile([128, 2 * K * D], f32, tag="rn")
            nc.sync.dma_start(out=rn_sb, in_=rn_ap[4 * j : 4 * j + 4])
            rel_sb = rn_sb[:, 0 : K * D].rearrange("p (k d) -> p k d", k=K)

            side4 = side_all[:, j, :]
            side4_bk = bass.AP(
                tensor=side4.tensor,
                offset=side4.offset,
                ap=[side4.ap[0], [0, K], side4.ap[-1]],
            )

            # scores[p, k] = sum_d rel[p,k,d] * side[b(p),d]
            prod = ps_prod.tile([128, K, D], f32, tag="prod")
            nc.vector.tensor_mul(out=prod, in0=rel_sb, in1=side4_bk)
            scores = work.tile([128, K], f32, tag="scores")
            nc.vector.tensor_reduce(
                out=scores, in_=prod, axis=mybir.AxisListType.X, op=Alu.add
            )
            # exp on ACT (no max-subtraction: |scores| <~ 6*sqrt(64) stays well
            # inside the f32 exp range); row-sum on DVE so the downstream
            # reciprocal never waits on a cross-engine accumulator drain
            escores = work.tile([128, K], f32, tag="escores")
            nc.scalar.activation(out=escores, in_=scores, func=Act.Exp)
            st[j] = (rn_sb, escores)

        def stage_b(j):
            rn_sb, escores = st.pop(j)
            nbrt_sb = rn_sb[:, K * D : 2 * K * D].rearrange("p (d k) -> p d k", d=D)

            sums = work.tile([128, 1], f32, tag="sums")
            nc.vector.tensor_reduce(
                out=sums, in_=escores, axis=mybir.AxisListType.X, op=Alu.add
            )
            rsums = work.tile([128, 1], f32, tag="rsums")
            nc.vector.reciprocal(out=rsums, in_=sums)

            # agg_u[p, d] = sum_k escores[p,k] * nbrt[p,d,k]; then scale by 1/sum
            esc_bdk = bass.AP(
                tensor=escores.tensor,
                offset=escores.offset,
                ap=[escores.ap[0], [0, D], escores.ap[-1]],
            )
            prod2 = prods.tile([128, D, K], f32, tag="prod2")
            nc.vector.tensor_mul(out=prod2, in0=nbrt_sb, in1=esc_bdk)
            agg_u = work.tile([128, D], f32, tag="agg_u")
            nc.vector.tensor_reduce(
                out=agg_u, in_=prod2, axis=mybir.AxisListType.X, op=Alu.add
            )
            agg = work.tile([128, D], f32, tag="agg")
            nc.vector.tensor_scalar_mul(out=agg, in0=agg_u, scalar1=rsums)

            # linear: lin = self@W1 + agg@W2 + ones^T b (PSUM-accumulated)
            selft_sb = work.tile([D, 128], f32, tag="selft_sb")
            nc.sync.dma_start(out=selft_sb, in_=selft_ap[:, 4 * j : 4 * j + 4, :])
            at_ps = ps_at.tile([D, 128], f32, tag="at")
            nc.tensor.transpose(out=at_ps, in_=agg, identity=iden_sb)
            at_sb = work.tile([D, 128], f32, tag="at_sb")
            nc.scalar.copy(out=at_sb, in_=at_ps)
            lin_ps = ps_lin.tile([128, D], f32, tag="lin")
            nc.tensor.matmul(
                out=lin_ps, lhsT=selft_sb, rhs=w1_sb, start=True, stop=False
            )
            nc.tensor.matmul(
                out=lin_ps, lhsT=at_sb, rhs=w2_sb, start=False, stop=False
            )
            nc.tensor.matmul(
                out=lin_ps, lhsT=ones_sb, rhs=bvec_sb, start=False, stop=True
            )
            outb = work.tile([128, D], f32, tag="outb")
            nc.scalar.activation(out=outb, in_=lin_ps, func=Act.Relu)
            nc.sync.dma_start(out=out_ap[4 * j : 4 * j + 4], in_=outb)

        for j in range(NJ + 1):
            if j < NJ:
                stage_a(j)
            if j >= 1:
                stage_b(j - 1)

    nc.finalize()
    return nc


def _get_nc():
    if "nc" not in _CACHE:
        _CACHE["nc"] = _build_nc()
    return _CACHE["nc"]


def _make_in_maps(self_vectors, neighbor_vectors, neighbor_relations, side_embeddings, W, b):
    iden = np.eye(128, dtype=np.float32)
    ones = np.ones((1, 128), dtype=np.float32)
    rel = np.asarray(neighbor_relations, dtype=np.float32).reshape(B, E, K * D)
    nbrt = (
        np.asarray(neighbor_vectors, dtype=np.float32)
        .transpose(0, 1, 3, 2)
        .reshape(B, E, D * K)
    )
    rn = np.concatenate([rel, nbrt], axis=2)  # [B, E, 4096]
    sv = np.asarray(self_vectors, dtype=np.float32)
    in_maps = []
    for c in range(NCORES):
        sl = slice(c * BC, (c + 1) * BC)
        in_maps.append(
            {
                "rn": np.ascontiguousarray(rn[sl]),
                "selft": np.ascontiguousarray(sv[sl].transpose(2, 0, 1)),
                "side": np.ascontiguousarray(side_embeddings[sl], dtype=np.float32),
                "wmat": np.ascontiguousarray(W, dtype=np.float32),
                "bvec": np.ascontiguousarray(b, dtype=np.float32).reshape(1, D),
                "ones": ones,
                "iden": iden,
            }
        )
    return in_maps


def kernel(self_vectors, neighbor_vectors, neighbor_relations, side_embeddings, W, b,
           _trace=False, _tmpdir=None):
    from concourse import bass_utils

    nc = _get_nc()
    in_maps = _make_in_maps(
        self_vectors, neighbor_vectors, neighbor_relations, side_embeddings, W, b
    )
    res = bass_utils.run_bass_kernel_spmd(
        nc, in_maps, list(range(NCORES)), trace=_trace, tmpdir=_tmpdir
    )
    _CACHE["last_results"] = res
    out = np.concatenate([res.results[c]["out"] for c in range(NCORES)], axis=0)
    return out


# revision 26
# speedup vs baseline: 1.0106x; 1.0106x over previous
"""Trainium2 Bass kernel for nn_Aggregator (GNN message passing).

Computation per (b, e):
  scores[k] = <side[b], rel[b,e,k,:]>          (contract over D=64)
  attn      = softmax_k(scores)
  agg[d]    = sum_k attn[k] * nbr[b,e,k,d]     (contract over K=32)
  out       = relu(cat(self[b,e], agg) @ W + bias)

Sharding: data-parallel over the leading batch dim B=1024 across 8 cores
(128 batches/core); weights replicated.

Per-core mapping, fully unrolled over 32 "bgroups" of 4 batches with all
big tiles on (4b x 32e) partitions:
  - rel and nbr^T arrive host-packed in one tensor (one sequential DMA/j)
  - scores/agg are DVE broadcast-multiply + innermost-axis reduce pairs
  - softmax: ACT exp, DVE row-sum + reciprocal; the 1/sum is folded into
    a single post-scale of the unnormalized aggregate
  - linear: self^T arrives host-transposed; agg^T via one PE transpose;
    out = self@W1 + agg@W2 + 1x128 rank-1 bias, PSUM-accumulated on PE;
    relu on ACT
  - the loop body is software-pipelined: scores(j) is emitted before the
    softmax/agg/linear of j-1, so ACT/PE latency hides under DVE work
"""

import numpy as np

B, E, K, D = 1024, 32, 32, 64
NCORES = 8
BC = B // NCORES  # 128 batches per core
NJ = BC // 4      # 32 bgroups of 4 batches

_CACHE = {}


def _build_nc():
    from contextlib import ExitStack

    import concourse.bass as bass
    import concourse.bacc as bacc
    import concourse.tile as tile
    from concourse import mybir

    f32 = mybir.dt.float32
    Alu = mybir.AluOpType
    Act = mybir.ActivationFunctionType

    # Bacc (not raw Bass): its finalize() legalizes sync waits -- TRN2 allows
    # at most 1 wait per instruction; excess waits split into EventSemaphores.
    nc = bacc.Bacc()

    # rel and nbr^T packed along the last axis: rn[b,e,0:2048] = rel[b,e]
    # (k-major), rn[b,e,2048:4096] = nbr[b,e]^T (d-major) -- one DMA per j.
    rn_h = nc.declare_dram_parameter("rn", [BC, E, 2 * K * D], f32, isOutput=False)
    selft_h = nc.declare_dram_parameter("selft", [D, BC, E], f32, isOutput=False)
    side_h = nc.declare_dram_parameter("side", [BC, D], f32, isOutput=False)
    w_h = nc.declare_dram_parameter("wmat", [2 * D, D], f32, isOutput=False)
    b_h = nc.declare_dram_parameter("bvec", [1, D], f32, isOutput=False)
    ones_h = nc.declare_dram_parameter("ones", [1, 128], f32, isOutput=False)
    iden_h = nc.declare_dram_parameter("iden", [128, 128], f32, isOutput=False)
    out_h = nc.declare_dram_parameter("out", [BC, E, D], f32, isOutput=True)

    rn_ap = rn_h[:]
    selft_ap = selft_h[:]
    out_ap = out_h[:]

    with tile.TileContext(nc) as tc, ExitStack() as ctx:
        consts = ctx.enter_context(tc.tile_pool(name="consts", bufs=1))
        bigrn = ctx.enter_context(tc.tile_pool(name="bigrn", bufs=6))
        prods = ctx.enter_context(tc.tile_pool(name="prods", bufs=2))
        work = ctx.enter_context(tc.tile_pool(name="work", bufs=6))
        # scores intermediate in PSUM: DVE's PSUM ports are separate from its
        # SBUF ports
        ps_prod = ctx.enter_context(tc.tile_pool(name="ps_prod", bufs=1, space="PSUM"))
        ps_at = ctx.enter_context(tc.tile_pool(name="ps_at", bufs=2, space="PSUM"))
        ps_lin = ctx.enter_context(tc.tile_pool(name="ps_lin", bufs=2, space="PSUM"))

        w1_sb = consts.tile([D, D], f32)
        nc.sync.dma_start(out=w1_sb, in_=w_h[:][0:D])
        w2_sb = consts.tile([D, D], f32)
        nc.sync.dma_start(out=w2_sb, in_=w_h[:][D : 2 * D])
        bvec_sb = consts.tile([1, D], f32)
        nc.sync.dma_start(out=bvec_sb, in_=b_h[:])
        ones_sb = consts.tile([1, 128], f32)
        nc.sync.dma_start(out=ones_sb, in_=ones_h[:])
        iden_sb = consts.tile([128, 128], f32)
        nc.sync.dma_start(out=iden_sb, in_=iden_h[:])
        # side_all[p=(bg,e), j, d] = side[4j + bg, d]; loaded once, one DMA
        # per bg-row block (3-dim AP limit).
        side_all = consts.tile([128, NJ, D], f32)
        for bg in range(4):
            nc.sync.dma_start(
                out=side_all[32 * bg : 32 * bg + 32],
                in_=bass.AP(
                    tensor=side_h[:].tensor,
                    offset=bg * D,
                    ap=[[0, 32], [4 * D, NJ], [1, D]],
                ),
            )

        # pipeline state carried from stage A (scores) to stage B (rest)
        st = {}

        def stage_a(j):
            rn_sb = bigrn.tile([128, 2 * K * D], f32, tag="rn")
            nc.sync.dma_start(out=rn_sb, in_=rn_ap[4 * j : 4 * j + 4])
            rel_sb = rn_sb[:, 0 : K * D].rearrange("p (k d) -> p k d", k=K)

            side4 = side_all[:, j, :]
            side4_bk = bass.AP(
                tensor=side4.tensor,
                offset=side4.offset,
                ap=[side4.ap[0], [0, K], side4.ap[-1]],
            )

            # scores[p, k] = sum_d rel[p,k,d] * side[b(p),d]
            prod = ps_prod.tile([128, K, D], f32, tag="prod")
            nc.vector.tensor_mul(out=prod, in0=rel_sb, in1=side4_bk)
            scores = work.tile([128, K], f32, tag="scores")
            nc.vector.tensor_reduce(
                out=scores, in_=prod, axis=mybir.AxisListType.X, op=Alu.add
            )
            # exp on ACT (no max-subtraction: |scores| <~ 6*sqrt(64) stays well
            # inside the f32 exp range); row-sum on DVE so the downstream
            # reciprocal never waits on a cross-engine accumulator drain
            escores = work.tile([128, K], f32, tag="escores")
            nc.scalar.activation(out=escores, in_=scores, func=Act.Exp)
            st[j] = (rn_sb, escores)

        def stage_b(j):
            rn_sb, escores = st.pop(j)
            nbrt_sb = rn_sb[:, K * D : 2 * K * D].rearrange("p (d k) -> p d k", d=D)

            sums = work.tile([128, 1], f32, tag="sums")
            nc.vector.tensor_reduce(
                out=sums, in_=escores, axis=mybir.AxisListType.X, op=Alu.add
            )
            rsums = work.tile([128, 1], f32, tag="rsums")
            nc.vector.reciprocal(out=rsums, in_=sums)

            # agg_u[p, d] = sum_k escores[p,k] * nbrt[p,d,k]; then scale by 1/sum
            esc_bdk = bass.AP(
                tensor=escores.tensor,
                offset=escores.offset,
                ap=[escores.ap[0], [0, D], escores.ap[-1]],
            )
            prod2 = prods.tile([128, D, K], f32, tag="prod2")
            nc.vector.tensor_mul(out=prod2, in0=nbrt_sb, in1=esc_bdk)
            agg_u = work.tile([128, D], f32, tag="agg_u")
            nc.vector.tensor_reduce(
                out=agg_u, in_=prod2, axis=mybir.AxisListType.X, op=Alu.add
            )
            agg = work.tile([128, D], f32, tag="agg")
            nc.vector.tensor_scalar_mul(out=agg, in0=agg_u, scalar1=rsums)

            # linear: lin = self@W1 + agg@W2 + ones^T b (PSUM-accumulated)
            selft_sb = work.tile([D, 128], f32, tag="selft_sb")
            nc.sync.dma_start(out=selft_sb, in_=selft_ap[:, 4 * j : 4 * j + 4, :])
            at_ps = ps_at.tile([D, 128], f32, tag="at")
            nc.tensor.transpose(out=at_ps, in_=agg, identity=iden_sb)
            at_sb = work.tile([D, 128], f32, tag="at_sb")
            nc.scalar.copy(out=at_sb, in_=at_ps)
            lin_ps = ps_lin.tile([128, D], f32, tag="lin")
            nc.tensor.matmul(
                out=lin_ps, lhsT=selft_sb, rhs=w1_sb, start=True, stop=False
            )
            nc.tensor.matmul(
                out=lin_ps, lhsT=at_sb, rhs=w2_sb, start=False, stop=False
            )
            nc.tensor.matmul(
                out=lin_ps, lhsT=ones_sb, rhs=bvec_sb, start=False, stop=True
            )
            outb = work.tile([128, D], f32, tag="outb")
            nc.scalar.activation(out=outb, in_=lin_ps, func=Act.Relu)
            nc.sync.dma_start(out=out_ap[4 * j : 4 * j + 4], in_=outb)

        # depth-2 pipeline: stage_b runs two iterations behind stage_a so the
        # ACT exp latency is covered by two full scores phases of DVE work
        for j in range(NJ + 2):
            if j < NJ:
                stage_a(j)
            if j >= 2:
                stage_b(j - 2)

    nc.finalize()
    return nc


def _get_nc():
    if "nc" not in _CACHE:
        _CACHE["nc"] = _build_nc()
    return _CACHE["nc"]


def _make_in_maps(self_vectors, neighbor_vectors, neighbor_relations, side_embeddings, W, b):
    iden = np.eye(128, dtype=np.float32)
    ones = np.ones((1, 128), dtype=np.float32)
    rel = np.asarray(neighbor_relations, dtype=np.float32).reshape(B, E, K * D)
    nbrt = (
        np.asarray(neighbor_vectors, dtype=np.float32)
        .transpose(0, 1, 3, 2)
        .reshape(B, E, D * K)
    )
    rn = np.concatenate([rel, nbrt], axis=2)  # [B, E, 4096]
    sv = np.asarray(self_vectors, dtype=np.float32)
    in_maps = []
    for c in range(NCORES):
        sl = slice(c * BC, (c + 1) * BC)
        in_maps.append(
            {
                "rn": np.ascontiguousarray(rn[sl]),
                "selft": np.ascontiguousarray(sv[sl].transpose(2, 0, 1)),
                "side": np.ascontiguousarray(side_embeddings[sl], dtype=np.float32),
                "wmat": np.ascontiguousarray(W, dtype=np.float32),
                "bvec": np.ascontiguousarray(b, dtype=np.float32).reshape(1, D),
                "ones": ones,
                "iden": iden,
            }
        )
    return in_maps


def kernel(self_vectors, neighbor_vectors, neighbor_relations, side_embeddings, W, b,
           _trace=False, _tmpdir=None):
    from concourse import bass_utils

    nc = _get_nc()
    in_maps = _make_in_maps(
        self_vectors, neighbor_vectors, neighbor_relations, side_embeddings, W, b
    )
    res = bass_utils.run_bass_kernel_spmd(
        nc, in_maps, list(range(NCORES)), trace=_trace, tmpdir=_tmpdir
    )
    _CACHE["last_results"] = res
    out = np.concatenate([res.results[c]["out"] for c in range(NCORES)], axis=0)
    return out
